# revision 38
# baseline (speedup 1.0000x reference)
"""Chamfer distance kernel for Trainium2 (8 NeuronCores, SPMD) — v3.

Problem: pred [2, 8192, 3], gt [2, 8192, 3] (fp32) ->
  scalar = mean_b( mean_i min_j ||pred[b,j]-gt[b,i]|| + mean_j min_i ||...|| )

Strategy per core (gt rows sharded 8-way):
  d2[i,j] = g2_i + p2_j - 2<g_i, p_j> as a K=16 fp16 matmul (hi/lo
  compensated, see _pack_inputs).  The S (gt-side) matrix is NEGATED on the
  host so the PE produces -d2; every min becomes a max, which lets the
  column (partition-axis) reduction use gpsimd.partition_all_reduce(max).

  Measured engine rates drive the op choice (per free-elem per partition):
  tensor_tensor fp16 runs at 2x (0.52ns); every reduce-class op
  (tensor_reduce / vector.max / tensor_scalar+accum) is 1x (1.04ns);
  activation copy is 0.83ns.  gpsimd partition_all_reduce works but
  starves concurrent DVE ops (~20us stalls), so it is off by default.
  scalar_tensor_tensor / tensor_tensor_reduce with max wedge the device.

  Per (b, g-block of 128 gt rows): PE fills [128, 2048] fp32 PSUM tiles
  (4x512 matmuls, 2 rotating PSUM slots); ScalarE evacuates them to a
  [128, 8192] fp16 row buffer (the only engine free to do so - gpsimd
  cannot read PSUM).  DVE does all max work at 2x: a tensor_tensor
  halving tree for the row max (8192->512, L4 lands in a per-batch
  [128, 8, 512] buffer finished by one batched 1x reduce), and one wide
  tensor_tensor fold of the row buffer into the per-batch column
  accumulator.  Each batch's partition-axis tail: PE transposes the
  accumulator in 128-wide chunks into PSUM and DVE max-reduces.
"""

import os
import sys

import numpy as np

for _p in ("/opt/trn_rl_repo", os.path.expanduser("~/.axon_site/_ro/trn_rl_repo")):
    if os.path.isdir(_p) and _p not in sys.path:
        sys.path.insert(0, _p)
        break

import concourse.bacc as bacc
import concourse.bass as bass
import concourse.tile as tile
from concourse import bass_isa, mybir
from concourse.masks import make_identity

FP32 = mybir.dt.float32
FP16 = mybir.dt.float16
MAX = mybir.AluOpType.max
NEG_INF = -1e30

N_CORES = 8
B = 2
N = 8192
OWN = N // N_CORES  # 1024 gt rows per core per batch
GBLK = OWN // 128   # 8 blocks of 128 gt rows
CHW = 2048          # PSUM cast-tile width (4 banks; 2 rotating slots)
NCH = N // CHW      # 4 tiles per row block
MMW = 512           # matmul free width (one PSUM bank)

PACKED = int(os.environ.get("CHAMFER_PACKED", "4"))
WARMUP = int(os.environ.get("CHAMFER_WARMUP", "8"))
# b0/b1 tail modes: "par" (gpsimd partition_all_reduce) or "tr" (PE+DVE)
TAIL0 = os.environ.get("CHAMFER_TAIL0", "tr")
TAIL1 = os.environ.get("CHAMFER_TAIL1", "tr")


def build_nc():
    nc = bacc.Bacc()
    s_in = nc.dram_tensor("s_in", [128, B, OWN], FP16, kind="ExternalInput")
    t_in = nc.dram_tensor("t_in", [128, B, N], FP16, kind="ExternalInput")
    rowmax_out = nc.dram_tensor("rowmax_out", [128, B, GBLK], FP32, kind="ExternalOutput")
    colmax_par = nc.dram_tensor("colmax_par", [B, N], FP16, kind="ExternalOutput")
    colmax_tr = nc.dram_tensor("colmax_tr", [B, 128, N // 128], FP32, kind="ExternalOutput")

    with tile.TileContext(nc) as tc:
        with (
            tc.tile_pool(name="consts", bufs=1) as consts,
            tc.tile_pool(name="psum", bufs=2, space="PSUM") as psum,
            tc.tile_pool(name="rowbufs", bufs=3) as rowbufs,
            tc.tile_pool(name="tree", bufs=2) as treep,
            tc.tile_pool(name="coll", bufs=1) as coll,
        ):
            t_sb = consts.tile([128, B, N], FP16, tag="t_sb")
            s_sb = consts.tile([128, B, OWN], FP16, tag="s_sb")
            nc.sync.dma_start(out=s_sb[:, 0], in_=s_in[:, 0])
            nc.sync.dma_start(out=t_sb[:, 0, :CHW], in_=t_in[:, 0, :CHW])
            nc.sync.dma_start(out=t_sb[:, 0, CHW:], in_=t_in[:, 0, CHW:])
            nc.sync.dma_start(out=s_sb[:, 1], in_=s_in[:, 1])
            nc.sync.dma_start(out=t_sb[:, 1], in_=t_in[:, 1])

            colacc = [
                [coll.tile([128, N], FP16, name=f"ca_{b}", tag=f"ca_{b}")]
                for b in range(B)
            ]
            parout = coll.tile([128, N], FP16, tag="parout")
            rowmax_coll = coll.tile([128, B * GBLK], FP32, tag="rowmax_coll")
            ident = consts.tile([128, 128], FP16, tag="ident")
            make_identity(nc, ident)
            colmax_sb = coll.tile([128, B, N // 128], FP32, tag="colmax_sb")

            warm_in = consts.tile([128, MMW], FP16, tag="warm_in")
            nc.vector.memset(warm_in, 0.0)
            wps = psum.tile([128, CHW], FP32, name="wps", tag="ps")
            for i in range(WARMUP):
                nc.tensor.matmul(
                    out=wps[:, (i % NCH) * MMW : (i % NCH + 1) * MMW],
                    lhsT=warm_in[:, 0:128],
                    rhs=warm_in[:, 0:MMW],
                    start=True,
                    stop=True,
                )

            def emit_par(acc, slot):
                nc.gpsimd.partition_all_reduce(
                    out_ap=parout,
                    in_ap=acc,
                    channels=128,
                    reduce_op=bass_isa.ReduceOp.max,
                )
                nc.sync.dma_start(out=colmax_par[slot], in_=parout[0:1, :])

            def emit_tr(acc, b):
                # PE transposes 128-chunks into PSUM, DVE max-reduces.
                # colmax_sb[p, b, s*16+c] = max for pred s*2048 + c*128 + p
                for s in range(2):
                    tp = psum.tile([128, 2 * CHW], FP16, name="tp", tag="ps")
                    for c in range(32):
                        nc.tensor.transpose(
                            out=tp[:, c * 128 : (c + 1) * 128],
                            in_=acc[:, s * 2 * CHW + c * 128 : s * 2 * CHW + (c + 1) * 128],
                            identity=ident,
                        )
                    nc.vector.tensor_reduce(
                        out=colmax_sb[:, b, s * 32 : (s + 1) * 32],
                        in_=tp.rearrange("p (c q) -> p c q", q=128),
                        axis=mybir.AxisListType.X,
                        op=MAX,
                    )
                    nc.sync.dma_start(
                        out=colmax_tr[b, :, s * 32 : (s + 1) * 32],
                        in_=colmax_sb[:, b, s * 32 : (s + 1) * 32],
                    )

            rowtree = coll.tile([128, B, GBLK, 512], FP16, tag="rowtree")
            qi = 0
            for b in range(B):
                for g in range(GBLK):
                    acc = colacc[b][0]
                    dst_row = acc if g == 0 else rowbufs.tile(
                        [128, N], FP16, tag="rowbuf"
                    )
                    for ch in range(NCH):
                        ps = psum.tile([128, CHW], FP32, tag="ps")
                        for q in range(NCH):
                            strip = (qi % PACKED) * 32 if PACKED > 1 else 0
                            qi += 1
                            nc.tensor.matmul(
                                out=ps[:, q * MMW : (q + 1) * MMW],
                                lhsT=s_sb[
                                    strip : strip + 32, b, g * 128 : (g + 1) * 128
                                ],
                                rhs=t_sb[
                                    strip : strip + 32,
                                    b,
                                    ch * CHW + q * MMW : ch * CHW + (q + 1) * MMW,
                                ],
                                start=True,
                                stop=True,
                                tile_position=(strip, 0) if PACKED > 1 else None,
                            )
                        if b == 0 and g == 0 and ch >= 2:
                            # DVE is idle during the ramp; splitting the
                            # first region's casts shortens the pipe fill
                            nc.vector.tensor_scalar(
                                out=dst_row[:, ch * CHW : (ch + 1) * CHW],
                                in0=ps, scalar1=NEG_INF, scalar2=None, op0=MAX,
                            )
                        else:
                            nc.scalar.activation(
                                out=dst_row[:, ch * CHW : (ch + 1) * CHW],
                                in_=ps,
                                func=mybir.ActivationFunctionType.Copy,
                            )
                    if g != 0:
                        nc.vector.tensor_tensor(
                            out=acc, in0=acc, in1=dst_row, op=MAX
                        )
                    if b > 0 and g == 1:
                        # previous batch's tail lands here so its PSUM-slot
                        # use is deprioritized below this batch's first block
                        emit_tr(colacc[b - 1][0], b - 1)
                    if b == B - 1 and g == GBLK - 1:
                        emit_tr(acc, b)
                    # row max: 2x halving tree into the per-batch 512 buffer
                    ta = treep.tile([128, 4096], FP16, tag="ta")
                    tb = treep.tile([128, 2048], FP16, tag="tb")
                    nc.vector.tensor_tensor(
                        out=ta, in0=dst_row[:, :4096], in1=dst_row[:, 4096:], op=MAX
                    )
                    nc.vector.tensor_tensor(
                        out=tb, in0=ta[:, :2048], in1=ta[:, 2048:], op=MAX
                    )
                    nc.vector.tensor_tensor(
                        out=ta[:, :1024], in0=tb[:, :1024], in1=tb[:, 1024:], op=MAX
                    )
                    nc.vector.tensor_tensor(
                        out=rowtree[:, b, g], in0=ta[:, :512], in1=ta[:, 512:1024],
                        op=MAX,
                    )
                # finish all 8 row maxes: two 2x halvings + one 1x reduce
                nc.vector.tensor_tensor(
                    out=rowtree[:, b, :, :256],
                    in0=rowtree[:, b, :, :256], in1=rowtree[:, b, :, 256:], op=MAX,
                )
                nc.vector.tensor_tensor(
                    out=rowtree[:, b, :, :128],
                    in0=rowtree[:, b, :, :128], in1=rowtree[:, b, :, 128:256], op=MAX,
                )
                nc.vector.tensor_reduce(
                    out=rowmax_coll[:, b * GBLK : (b + 1) * GBLK],
                    in_=rowtree[:, b, :, :128],
                    axis=mybir.AxisListType.X,
                    op=MAX,
                )
            nc.sync.dma_start(out=rowmax_out[:], in_=rowmax_coll.rearrange(
                "p (b g) -> p b g", b=B))
    nc.finalize()
    return nc


def _split_hl(x: np.ndarray):
    """fp32 -> (hi, lo) float16 pair with x ~= hi + lo."""
    hi = x.astype(np.float16)
    lo = (x - hi.astype(np.float32)).astype(np.float16)
    return hi, lo


def _pack_inputs(pred: np.ndarray, gt: np.ndarray, kp: int = 16):
    """Host-side shard prep: compensated hi/lo fp16 extended matrices.

    d2[i,j] = g2_i + p2_j - 2<g_i, p_j> is evaluated as a K=16 fp16 matmul
    with fp32 PSUM accumulation; each fp32 operand is split hi+lo and the
    three cross products (hi*hi, lo*hi, hi*lo) are packed into the K rows,
    so the only dropped term is lo*lo (~2^-22 relative).  The S side is
    negated so the device computes -d2 (mins become maxes).
    """
    pred = np.asarray(pred, dtype=np.float32)
    gt = np.asarray(gt, dtype=np.float32)
    bs, ng, _ = gt.shape
    _, npr, _ = pred.shape
    g2 = np.sum(gt * gt, axis=-1)
    p2 = np.sum(pred * pred, axis=-1)
    m = -2.0 * gt
    g2h, g2l = _split_hl(g2)
    p2h, p2l = _split_hl(p2)
    mh, ml = _split_hl(m)
    ph, pl = _split_hl(pred)

    s_full = np.zeros((kp, bs, ng), dtype=np.float16)
    t_full = np.zeros((kp, bs, npr), dtype=np.float16)
    s_full[0], t_full[0] = g2h, 1.0
    s_full[1], t_full[1] = g2l, 1.0
    s_full[2], t_full[2] = 1.0, p2h
    s_full[3], t_full[3] = 1.0, p2l
    for d in range(3):
        s_full[4 + d], t_full[4 + d] = mh[..., d], ph[..., d]
        s_full[7 + d], t_full[7 + d] = ml[..., d], ph[..., d]
        s_full[10 + d], t_full[10 + d] = mh[..., d], pl[..., d]
    s_full = -s_full  # device computes -d2; mins become maxes
    s_rep = np.zeros((128, bs, ng), dtype=np.float16)
    t_rep = np.zeros((128, bs, npr), dtype=np.float16)
    for q in range(4):
        s_rep[32 * q : 32 * q + kp] = s_full
        t_rep[32 * q : 32 * q + kp] = t_full
    return s_rep, t_rep


_NC_CACHE = {}


def _get_nc():
    key = (PACKED, WARMUP, TAIL0, TAIL1)
    if key not in _NC_CACHE:
        _NC_CACHE[key] = build_nc()
    return _NC_CACHE[key]


def _run_device(s_full, t_full, run_kwargs=None):
    from concourse.bass_utils import run_bass_kernel_spmd

    nc = _get_nc()
    in_maps = [
        {
            "s_in": np.ascontiguousarray(s_full[:, :, c * OWN : (c + 1) * OWN]),
            "t_in": t_full,
        }
        for c in range(N_CORES)
    ]
    res = run_bass_kernel_spmd(
        nc, in_maps, core_ids=list(range(N_CORES)), **(run_kwargs or {})
    )
    return res


def _combine(results):
    neg_d1 = np.empty((B, N), dtype=np.float32)
    colmaxes = []
    for c, out in enumerate(results):
        rm = out["rowmax_out"].transpose(1, 2, 0).reshape(B, OWN)
        neg_d1[:, c * OWN : (c + 1) * OWN] = rm
        cms = []
        for b in range(B):
            if (TAIL0 if b == 0 else TAIL1) == "par":
                cms.append(out["colmax_par"][b].astype(np.float32))
            else:
                # colmax_tr [b, 128(p), 64]: pred idx = col*128 + p
                cms.append(out["colmax_tr"][b].transpose(1, 0).reshape(N))
        colmaxes.append(np.stack(cms))
    neg_d2 = np.max(np.stack(colmaxes, axis=0), axis=0)
    d1 = np.sqrt(np.maximum(-neg_d1.astype(np.float64), 0.0))
    d2 = np.sqrt(np.maximum(-neg_d2.astype(np.float64), 0.0))
    val = np.mean(np.mean(d1, axis=1) + np.mean(d2, axis=1))
    return np.float32(val)


def kernel(pred: np.ndarray, gt: np.ndarray) -> np.ndarray:
    s_full, t_full = _pack_inputs(pred, gt)
    res = _run_device(s_full, t_full)
    return _combine(res.results)


# revision 40
# speedup vs baseline: 1.1967x; 1.1967x over previous
"""Chamfer distance kernel for Trainium2 (8 NeuronCores, SPMD) — v3.

Problem: pred [2, 8192, 3], gt [2, 8192, 3] (fp32) ->
  scalar = mean_b( mean_i min_j ||pred[b,j]-gt[b,i]|| + mean_j min_i ||...|| )

Strategy per core (gt rows sharded 8-way):
  d2[i,j] = g2_i + p2_j - 2<g_i, p_j> as a K=16 fp16 matmul (hi/lo
  compensated, see _pack_inputs).  The S (gt-side) matrix is NEGATED on the
  host so the PE produces -d2; every min becomes a max, which lets the
  column (partition-axis) reduction use gpsimd.partition_all_reduce(max).

  Measured engine rates drive the op choice (per free-elem per partition):
  tensor_tensor fp16 runs at 2x (0.52ns); every reduce-class op
  (tensor_reduce / vector.max / tensor_scalar+accum) is 1x (1.04ns);
  activation copy is 0.83ns.  gpsimd partition_all_reduce works but
  starves concurrent DVE ops (~20us stalls), so it is off by default.
  scalar_tensor_tensor / tensor_tensor_reduce with max wedge the device.

  Per (b, g-block of 128 gt rows): PE fills [128, 2048] fp32 PSUM tiles
  (4x512 matmuls, 2 rotating PSUM slots); ScalarE evacuates them to a
  [128, 8192] fp16 row buffer (the only engine free to do so - gpsimd
  cannot read PSUM).  DVE does all max work at 2x: a tensor_tensor
  halving tree for the row max (8192->512, L4 lands in a per-batch
  [128, 8, 512] buffer finished by one batched 1x reduce), and one wide
  tensor_tensor fold of the row buffer into the per-batch column
  accumulator.  Each batch's partition-axis tail: PE transposes the
  accumulator in 128-wide chunks into PSUM and DVE max-reduces.
"""

import os
import sys

import numpy as np

for _p in ("/opt/trn_rl_repo", os.path.expanduser("~/.axon_site/_ro/trn_rl_repo")):
    if os.path.isdir(_p) and _p not in sys.path:
        sys.path.insert(0, _p)
        break

import concourse.bacc as bacc
import concourse.bass as bass
import concourse.tile as tile
from concourse import bass_isa, mybir
from concourse.masks import make_identity

FP32 = mybir.dt.float32
FP16 = mybir.dt.float16
MAX = mybir.AluOpType.max
NEG_INF = -1e30

N_CORES = 8
B = 2
N = 8192
OWN = N // N_CORES  # 1024 gt rows per core per batch
GBLK = OWN // 128   # 8 blocks of 128 gt rows
CHW = 2048          # PSUM cast-tile width (4 banks; 2 rotating slots)
NCH = N // CHW      # 4 tiles per row block
MMW = 512           # matmul free width (one PSUM bank)

PACKED = int(os.environ.get("CHAMFER_PACKED", "4"))
WARMUP = int(os.environ.get("CHAMFER_WARMUP", "8"))
# b0/b1 tail modes: "par" (gpsimd partition_all_reduce) or "tr" (PE+DVE)
TAIL0 = os.environ.get("CHAMFER_TAIL0", "tr")
TAIL1 = os.environ.get("CHAMFER_TAIL1", "tr")


def build_nc():
    nc = bacc.Bacc()
    s_in = nc.dram_tensor("s_in", [128, B, OWN], FP16, kind="ExternalInput")
    t_in = nc.dram_tensor("t_in", [128, B, N], FP16, kind="ExternalInput")
    rowmax_out = nc.dram_tensor("rowmax_out", [128, B, GBLK], FP32, kind="ExternalOutput")
    colmax_par = nc.dram_tensor("colmax_par", [B, N], FP16, kind="ExternalOutput")
    colmax_tr = nc.dram_tensor("colmax_tr", [B, 128, N // 128], FP32, kind="ExternalOutput")

    with tile.TileContext(nc) as tc:
        with (
            tc.tile_pool(name="consts", bufs=1) as consts,
            tc.tile_pool(name="psum", bufs=2, space="PSUM") as psum,
            tc.tile_pool(name="rowbufs", bufs=3) as rowbufs,
            tc.tile_pool(name="tree", bufs=2) as treep,
            tc.tile_pool(name="coll", bufs=1) as coll,
        ):
            t_sb = consts.tile([128, B, N], FP16, tag="t_sb")
            s_sb = consts.tile([128, B, OWN], FP16, tag="s_sb")
            nc.sync.dma_start(out=s_sb[:, 0], in_=s_in[:, 0])
            nc.sync.dma_start(out=t_sb[:, 0, :CHW], in_=t_in[:, 0, :CHW])
            nc.sync.dma_start(out=t_sb[:, 0, CHW:], in_=t_in[:, 0, CHW:])
            nc.sync.dma_start(out=s_sb[:, 1], in_=s_in[:, 1])
            nc.sync.dma_start(out=t_sb[:, 1], in_=t_in[:, 1])

            colacc = [
                [coll.tile([128, N], FP16, name=f"ca_{b}", tag=f"ca_{b}")]
                for b in range(B)
            ]
            parout = coll.tile([128, N], FP16, tag="parout")
            rowmax_coll = coll.tile([128, B * GBLK], FP32, tag="rowmax_coll")
            ident = consts.tile([128, 128], FP16, tag="ident")
            make_identity(nc, ident)
            colmax_sb = coll.tile([128, B, N // 128], FP32, tag="colmax_sb")

            warm_in = consts.tile([128, MMW], FP16, tag="warm_in")
            nc.vector.memset(warm_in, 0.0)
            wps = psum.tile([128, CHW], FP32, name="wps", tag="ps")
            for i in range(WARMUP):
                nc.tensor.matmul(
                    out=wps[:, (i % NCH) * MMW : (i % NCH + 1) * MMW],
                    lhsT=warm_in[:, 0:128],
                    rhs=warm_in[:, 0:MMW],
                    start=True,
                    stop=True,
                )

            def emit_par(acc, slot):
                nc.gpsimd.partition_all_reduce(
                    out_ap=parout,
                    in_ap=acc,
                    channels=128,
                    reduce_op=bass_isa.ReduceOp.max,
                )
                nc.sync.dma_start(out=colmax_par[slot], in_=parout[0:1, :])

            def emit_tr(acc, b):
                # PE transposes 128-chunks into PSUM, DVE max-reduces.
                # colmax_sb[p, b, s*16+c] = max for pred s*2048 + c*128 + p
                for s in range(2):
                    tp = psum.tile([128, 2 * CHW], FP16, name="tp", tag="ps")
                    for c in range(32):
                        nc.tensor.transpose(
                            out=tp[:, c * 128 : (c + 1) * 128],
                            in_=acc[:, s * 2 * CHW + c * 128 : s * 2 * CHW + (c + 1) * 128],
                            identity=ident,
                        )
                    nc.vector.tensor_reduce(
                        out=colmax_sb[:, b, s * 32 : (s + 1) * 32],
                        in_=tp.rearrange("p (c q) -> p c q", q=128),
                        axis=mybir.AxisListType.X,
                        op=MAX,
                    )
                nc.sync.dma_start(out=colmax_tr[b], in_=colmax_sb[:, b])

            rowtree = coll.tile([128, B, GBLK, 1024], FP16, tag="rowtree")
            qi = 0
            for b in range(B):
                for g in range(GBLK):
                    acc = colacc[b][0]
                    dst_row = acc if g == 0 else rowbufs.tile(
                        [128, N], FP16, tag="rowbuf"
                    )
                    for ch in range(NCH):
                        ps = psum.tile([128, CHW], FP32, tag="ps")
                        for q in range(NCH):
                            strip = (qi % PACKED) * 32 if PACKED > 1 else 0
                            qi += 1
                            nc.tensor.matmul(
                                out=ps[:, q * MMW : (q + 1) * MMW],
                                lhsT=s_sb[
                                    strip : strip + 32, b, g * 128 : (g + 1) * 128
                                ],
                                rhs=t_sb[
                                    strip : strip + 32,
                                    b,
                                    ch * CHW + q * MMW : ch * CHW + (q + 1) * MMW,
                                ],
                                start=True,
                                stop=True,
                                tile_position=(strip, 0) if PACKED > 1 else None,
                            )
                        if b == 0 and g == 0 and ch >= 2:
                            # DVE is idle during the ramp; splitting the
                            # first region's casts shortens the pipe fill
                            nc.vector.tensor_scalar(
                                out=dst_row[:, ch * CHW : (ch + 1) * CHW],
                                in0=ps, scalar1=NEG_INF, scalar2=None, op0=MAX,
                            )
                        else:
                            nc.scalar.activation(
                                out=dst_row[:, ch * CHW : (ch + 1) * CHW],
                                in_=ps,
                                func=mybir.ActivationFunctionType.Copy,
                            )
                    if g != 0:
                        nc.vector.tensor_tensor(
                            out=acc, in0=acc, in1=dst_row, op=MAX
                        )
                    if b > 0 and g == 1:
                        # previous batch's tail lands here so its PSUM-slot
                        # use is deprioritized below this batch's first block
                        emit_tr(colacc[b - 1][0], b - 1)
                    if b == B - 1 and g == GBLK - 1:
                        emit_tr(acc, b)
                    # row max: 2x halving tree into the per-batch 512 buffer
                    ta = treep.tile([128, 4096], FP16, tag="ta")
                    tb = treep.tile([128, 2048], FP16, tag="tb")
                    nc.vector.tensor_tensor(
                        out=ta, in0=dst_row[:, :4096], in1=dst_row[:, 4096:], op=MAX
                    )
                    nc.vector.tensor_tensor(
                        out=tb, in0=ta[:, :2048], in1=ta[:, 2048:], op=MAX
                    )
                    nc.vector.tensor_tensor(
                        out=rowtree[:, b, g], in0=tb[:, :1024], in1=tb[:, 1024:],
                        op=MAX,
                    )
                # finish all 8 row maxes: two 2x halvings + one 1x reduce
                nc.vector.tensor_tensor(
                    out=rowtree[:, b, :, :512],
                    in0=rowtree[:, b, :, :512], in1=rowtree[:, b, :, 512:], op=MAX,
                )
                nc.vector.tensor_tensor(
                    out=rowtree[:, b, :, :256],
                    in0=rowtree[:, b, :, :256], in1=rowtree[:, b, :, 256:512], op=MAX,
                )
                nc.vector.tensor_tensor(
                    out=rowtree[:, b, :, :128],
                    in0=rowtree[:, b, :, :128], in1=rowtree[:, b, :, 128:256], op=MAX,
                )
                nc.vector.tensor_reduce(
                    out=rowmax_coll[:, b * GBLK : (b + 1) * GBLK],
                    in_=rowtree[:, b, :, :128],
                    axis=mybir.AxisListType.X,
                    op=MAX,
                )
            nc.sync.dma_start(out=rowmax_out[:], in_=rowmax_coll.rearrange(
                "p (b g) -> p b g", b=B))
    nc.finalize()
    return nc


def _split_hl(x: np.ndarray):
    """fp32 -> (hi, lo) float16 pair with x ~= hi + lo."""
    hi = x.astype(np.float16)
    lo = (x - hi.astype(np.float32)).astype(np.float16)
    return hi, lo


def _pack_inputs(pred: np.ndarray, gt: np.ndarray, kp: int = 16):
    """Host-side shard prep: compensated hi/lo fp16 extended matrices.

    d2[i,j] = g2_i + p2_j - 2<g_i, p_j> is evaluated as a K=16 fp16 matmul
    with fp32 PSUM accumulation; each fp32 operand is split hi+lo and the
    three cross products (hi*hi, lo*hi, hi*lo) are packed into the K rows,
    so the only dropped term is lo*lo (~2^-22 relative).  The S side is
    negated so the device computes -d2 (mins become maxes).
    """
    pred = np.asarray(pred, dtype=np.float32)
    gt = np.asarray(gt, dtype=np.float32)
    bs, ng, _ = gt.shape
    _, npr, _ = pred.shape
    g2 = np.sum(gt * gt, axis=-1)
    p2 = np.sum(pred * pred, axis=-1)
    m = -2.0 * gt
    g2h, g2l = _split_hl(g2)
    p2h, p2l = _split_hl(p2)
    mh, ml = _split_hl(m)
    ph, pl = _split_hl(pred)

    s_full = np.zeros((kp, bs, ng), dtype=np.float16)
    t_full = np.zeros((kp, bs, npr), dtype=np.float16)
    s_full[0], t_full[0] = g2h, 1.0
    s_full[1], t_full[1] = g2l, 1.0
    s_full[2], t_full[2] = 1.0, p2h
    s_full[3], t_full[3] = 1.0, p2l
    for d in range(3):
        s_full[4 + d], t_full[4 + d] = mh[..., d], ph[..., d]
        s_full[7 + d], t_full[7 + d] = ml[..., d], ph[..., d]
        s_full[10 + d], t_full[10 + d] = mh[..., d], pl[..., d]
    s_full = -s_full  # device computes -d2; mins become maxes
    s_rep = np.zeros((128, bs, ng), dtype=np.float16)
    t_rep = np.zeros((128, bs, npr), dtype=np.float16)
    for q in range(4):
        s_rep[32 * q : 32 * q + kp] = s_full
        t_rep[32 * q : 32 * q + kp] = t_full
    return s_rep, t_rep


_NC_CACHE = {}


def _get_nc():
    key = (PACKED, WARMUP, TAIL0, TAIL1)
    if key not in _NC_CACHE:
        _NC_CACHE[key] = build_nc()
    return _NC_CACHE[key]


def _run_device(s_full, t_full, run_kwargs=None):
    from concourse.bass_utils import run_bass_kernel_spmd

    nc = _get_nc()
    in_maps = [
        {
            "s_in": np.ascontiguousarray(s_full[:, :, c * OWN : (c + 1) * OWN]),
            "t_in": t_full,
        }
        for c in range(N_CORES)
    ]
    res = run_bass_kernel_spmd(
        nc, in_maps, core_ids=list(range(N_CORES)), **(run_kwargs or {})
    )
    return res


def _combine(results):
    neg_d1 = np.empty((B, N), dtype=np.float32)
    colmaxes = []
    for c, out in enumerate(results):
        rm = out["rowmax_out"].transpose(1, 2, 0).reshape(B, OWN)
        neg_d1[:, c * OWN : (c + 1) * OWN] = rm
        cms = []
        for b in range(B):
            if (TAIL0 if b == 0 else TAIL1) == "par":
                cms.append(out["colmax_par"][b].astype(np.float32))
            else:
                # colmax_tr [b, 128(p), 64]: pred idx = col*128 + p
                cms.append(out["colmax_tr"][b].transpose(1, 0).reshape(N))
        colmaxes.append(np.stack(cms))
    neg_d2 = np.max(np.stack(colmaxes, axis=0), axis=0)
    d1 = np.sqrt(np.maximum(-neg_d1.astype(np.float64), 0.0))
    d2 = np.sqrt(np.maximum(-neg_d2.astype(np.float64), 0.0))
    val = np.mean(np.mean(d1, axis=1) + np.mean(d2, axis=1))
    return np.float32(val)


def kernel(pred: np.ndarray, gt: np.ndarray) -> np.ndarray:
    s_full, t_full = _pack_inputs(pred, gt)
    res = _run_device(s_full, t_full)
    return _combine(res.results)


# revision 42
# speedup vs baseline: 1.2018x; 1.0042x over previous
"""Chamfer distance kernel for Trainium2 (8 NeuronCores, SPMD) — v3.

Problem: pred [2, 8192, 3], gt [2, 8192, 3] (fp32) ->
  scalar = mean_b( mean_i min_j ||pred[b,j]-gt[b,i]|| + mean_j min_i ||...|| )

Strategy per core (gt rows sharded 8-way):
  d2[i,j] = g2_i + p2_j - 2<g_i, p_j> as a K=16 fp16 matmul (hi/lo
  compensated, see _pack_inputs).  The S (gt-side) matrix is NEGATED on the
  host so the PE produces -d2; every min becomes a max, which lets the
  column (partition-axis) reduction use gpsimd.partition_all_reduce(max).

  Measured engine rates drive the op choice (per free-elem per partition):
  tensor_tensor fp16 runs at 2x (0.52ns); every reduce-class op
  (tensor_reduce / vector.max / tensor_scalar+accum) is 1x (1.04ns);
  activation copy is 0.83ns.  gpsimd partition_all_reduce works but
  starves concurrent DVE ops (~20us stalls), so it is off by default.
  scalar_tensor_tensor / tensor_tensor_reduce with max wedge the device.

  Per (b, g-block of 128 gt rows): PE fills [128, 2048] fp32 PSUM tiles
  (4x512 matmuls, 2 rotating PSUM slots); ScalarE evacuates them to a
  [128, 8192] fp16 row buffer (the only engine free to do so - gpsimd
  cannot read PSUM).  DVE does all max work at 2x: a tensor_tensor
  halving tree for the row max (8192->512, L4 lands in a per-batch
  [128, 8, 512] buffer finished by one batched 1x reduce), and one wide
  tensor_tensor fold of the row buffer into the per-batch column
  accumulator.  Each batch's partition-axis tail: PE transposes the
  accumulator in 128-wide chunks into PSUM and DVE max-reduces.
"""

import os
import sys

import numpy as np

for _p in ("/opt/trn_rl_repo", os.path.expanduser("~/.axon_site/_ro/trn_rl_repo")):
    if os.path.isdir(_p) and _p not in sys.path:
        sys.path.insert(0, _p)
        break

import concourse.bacc as bacc
import concourse.bass as bass
import concourse.tile as tile
from concourse import bass_isa, mybir
from concourse.masks import make_identity

FP32 = mybir.dt.float32
FP16 = mybir.dt.float16
MAX = mybir.AluOpType.max
NEG_INF = -1e30

N_CORES = 8
B = 2
N = 8192
OWN = N // N_CORES  # 1024 gt rows per core per batch
GBLK = OWN // 128   # 8 blocks of 128 gt rows
CHW = 2048          # PSUM cast-tile width (4 banks; 2 rotating slots)
NCH = N // CHW      # 4 tiles per row block
MMW = 512           # matmul free width (one PSUM bank)

PACKED = int(os.environ.get("CHAMFER_PACKED", "4"))
WARMUP = int(os.environ.get("CHAMFER_WARMUP", "8"))
# b0/b1 tail modes: "par" (gpsimd partition_all_reduce) or "tr" (PE+DVE)
TAIL0 = os.environ.get("CHAMFER_TAIL0", "tr")
TAIL1 = os.environ.get("CHAMFER_TAIL1", "tr")


def build_nc():
    nc = bacc.Bacc()
    s_in = nc.dram_tensor("s_in", [128, B, OWN], FP16, kind="ExternalInput")
    t_in = nc.dram_tensor("t_in", [128, B, N], FP16, kind="ExternalInput")
    rowmax_out = nc.dram_tensor("rowmax_out", [128, B, GBLK], FP32, kind="ExternalOutput")
    colmax_par = nc.dram_tensor("colmax_par", [B, N], FP16, kind="ExternalOutput")
    colmax_tr = nc.dram_tensor("colmax_tr", [B, 128, N // 128], FP32, kind="ExternalOutput")

    with tile.TileContext(nc) as tc:
        with (
            tc.tile_pool(name="consts", bufs=1) as consts,
            tc.tile_pool(name="psum", bufs=2, space="PSUM") as psum,
            tc.tile_pool(name="rowbufs", bufs=3) as rowbufs,
            tc.tile_pool(name="tree", bufs=2) as treep,
            tc.tile_pool(name="coll", bufs=1) as coll,
        ):
            t_sb = consts.tile([128, B, N], FP16, tag="t_sb")
            s_sb = consts.tile([128, B, OWN], FP16, tag="s_sb")
            # first wave split across three DMA rings (sync/scalar/vector
            # each issue on their own queue) so region 0's data lands ~3x
            # sooner than a single serialized queue would deliver it
            nc.sync.dma_start(out=s_sb[:, 0], in_=s_in[:, 0])
            nc.scalar.dma_start(out=t_sb[:, 0, :CHW], in_=t_in[:, 0, :CHW])
            nc.sync.dma_start(out=t_sb[:, 0, CHW : N // 2], in_=t_in[:, 0, CHW : N // 2])
            nc.scalar.dma_start(out=t_sb[:, 0, N // 2 :], in_=t_in[:, 0, N // 2 :])
            nc.sync.dma_start(out=s_sb[:, 1], in_=s_in[:, 1])
            nc.scalar.dma_start(out=t_sb[:, 1, : N // 2], in_=t_in[:, 1, : N // 2])
            nc.sync.dma_start(out=t_sb[:, 1, N // 2 :], in_=t_in[:, 1, N // 2 :])

            colacc = [
                [coll.tile([128, N], FP16, name=f"ca_{b}", tag=f"ca_{b}")]
                for b in range(B)
            ]
            parout = coll.tile([128, N], FP16, tag="parout")
            rowmax_coll = coll.tile([128, B * GBLK], FP32, tag="rowmax_coll")
            ident = consts.tile([128, 128], FP16, tag="ident")
            make_identity(nc, ident)
            colmax_sb = coll.tile([128, B, N // 128], FP32, tag="colmax_sb")

            warm_in = consts.tile([128, MMW], FP16, tag="warm_in")
            nc.vector.memset(warm_in, 0.0)
            wps = psum.tile([128, CHW], FP32, name="wps", tag="ps")
            for i in range(WARMUP):
                nc.tensor.matmul(
                    out=wps[:, (i % NCH) * MMW : (i % NCH + 1) * MMW],
                    lhsT=warm_in[:, 0:128],
                    rhs=warm_in[:, 0:MMW],
                    start=True,
                    stop=True,
                )

            def emit_par(acc, slot):
                nc.gpsimd.partition_all_reduce(
                    out_ap=parout,
                    in_ap=acc,
                    channels=128,
                    reduce_op=bass_isa.ReduceOp.max,
                )
                nc.sync.dma_start(out=colmax_par[slot], in_=parout[0:1, :])

            def emit_tr(acc, b):
                # PE transposes 128-chunks into PSUM, DVE max-reduces.
                # colmax_sb[p, b, s*16+c] = max for pred s*2048 + c*128 + p
                for s in range(2):
                    tp = psum.tile([128, 2 * CHW], FP16, name="tp", tag="ps")
                    for c in range(32):
                        nc.tensor.transpose(
                            out=tp[:, c * 128 : (c + 1) * 128],
                            in_=acc[:, s * 2 * CHW + c * 128 : s * 2 * CHW + (c + 1) * 128],
                            identity=ident,
                        )
                    nc.vector.tensor_reduce(
                        out=colmax_sb[:, b, s * 32 : (s + 1) * 32],
                        in_=tp.rearrange("p (c q) -> p c q", q=128),
                        axis=mybir.AxisListType.X,
                        op=MAX,
                    )
                nc.sync.dma_start(out=colmax_tr[b], in_=colmax_sb[:, b])

            rowtree = coll.tile([128, B, GBLK, 1024], FP16, tag="rowtree")
            qi = 0
            for b in range(B):
                for g in range(GBLK):
                    acc = colacc[b][0]
                    dst_row = acc if g == 0 else rowbufs.tile(
                        [128, N], FP16, tag="rowbuf"
                    )
                    for ch in range(NCH):
                        ps = psum.tile([128, CHW], FP32, tag="ps")
                        for q in range(NCH):
                            strip = (qi % PACKED) * 32 if PACKED > 1 else 0
                            qi += 1
                            nc.tensor.matmul(
                                out=ps[:, q * MMW : (q + 1) * MMW],
                                lhsT=s_sb[
                                    strip : strip + 32, b, g * 128 : (g + 1) * 128
                                ],
                                rhs=t_sb[
                                    strip : strip + 32,
                                    b,
                                    ch * CHW + q * MMW : ch * CHW + (q + 1) * MMW,
                                ],
                                start=True,
                                stop=True,
                                tile_position=(strip, 0) if PACKED > 1 else None,
                            )
                        if b == 0 and g == 0 and ch >= 2:
                            # DVE is idle during the ramp; splitting the
                            # first region's casts shortens the pipe fill
                            nc.vector.tensor_scalar(
                                out=dst_row[:, ch * CHW : (ch + 1) * CHW],
                                in0=ps, scalar1=NEG_INF, scalar2=None, op0=MAX,
                            )
                        else:
                            nc.scalar.activation(
                                out=dst_row[:, ch * CHW : (ch + 1) * CHW],
                                in_=ps,
                                func=mybir.ActivationFunctionType.Copy,
                            )
                    if g != 0:
                        nc.vector.tensor_tensor(
                            out=acc, in0=acc, in1=dst_row, op=MAX
                        )
                    if b > 0 and g == 1:
                        # previous batch's tail lands here so its PSUM-slot
                        # use is deprioritized below this batch's first block
                        emit_tr(colacc[b - 1][0], b - 1)
                    if b == B - 1 and g == GBLK - 1:
                        emit_tr(acc, b)
                    # row max: 2x halving tree into the per-batch 512 buffer
                    ta = treep.tile([128, 4096], FP16, tag="ta")
                    tb = treep.tile([128, 2048], FP16, tag="tb")
                    nc.vector.tensor_tensor(
                        out=ta, in0=dst_row[:, :4096], in1=dst_row[:, 4096:], op=MAX
                    )
                    nc.vector.tensor_tensor(
                        out=tb, in0=ta[:, :2048], in1=ta[:, 2048:], op=MAX
                    )
                    nc.vector.tensor_tensor(
                        out=rowtree[:, b, g], in0=tb[:, :1024], in1=tb[:, 1024:],
                        op=MAX,
                    )
                # finish all 8 row maxes: two 2x halvings + one 1x reduce
                nc.vector.tensor_tensor(
                    out=rowtree[:, b, :, :512],
                    in0=rowtree[:, b, :, :512], in1=rowtree[:, b, :, 512:], op=MAX,
                )
                nc.vector.tensor_tensor(
                    out=rowtree[:, b, :, :256],
                    in0=rowtree[:, b, :, :256], in1=rowtree[:, b, :, 256:512], op=MAX,
                )
                nc.vector.tensor_tensor(
                    out=rowtree[:, b, :, :128],
                    in0=rowtree[:, b, :, :128], in1=rowtree[:, b, :, 128:256], op=MAX,
                )
                nc.vector.tensor_reduce(
                    out=rowmax_coll[:, b * GBLK : (b + 1) * GBLK],
                    in_=rowtree[:, b, :, :128],
                    axis=mybir.AxisListType.X,
                    op=MAX,
                )
            nc.sync.dma_start(out=rowmax_out[:], in_=rowmax_coll.rearrange(
                "p (b g) -> p b g", b=B))
    nc.finalize()
    return nc


def _split_hl(x: np.ndarray):
    """fp32 -> (hi, lo) float16 pair with x ~= hi + lo."""
    hi = x.astype(np.float16)
    lo = (x - hi.astype(np.float32)).astype(np.float16)
    return hi, lo


def _pack_inputs(pred: np.ndarray, gt: np.ndarray, kp: int = 16):
    """Host-side shard prep: compensated hi/lo fp16 extended matrices.

    d2[i,j] = g2_i + p2_j - 2<g_i, p_j> is evaluated as a K=16 fp16 matmul
    with fp32 PSUM accumulation; each fp32 operand is split hi+lo and the
    three cross products (hi*hi, lo*hi, hi*lo) are packed into the K rows,
    so the only dropped term is lo*lo (~2^-22 relative).  The S side is
    negated so the device computes -d2 (mins become maxes).
    """
    pred = np.asarray(pred, dtype=np.float32)
    gt = np.asarray(gt, dtype=np.float32)
    bs, ng, _ = gt.shape
    _, npr, _ = pred.shape
    g2 = np.sum(gt * gt, axis=-1)
    p2 = np.sum(pred * pred, axis=-1)
    m = -2.0 * gt
    g2h, g2l = _split_hl(g2)
    p2h, p2l = _split_hl(p2)
    mh, ml = _split_hl(m)
    ph, pl = _split_hl(pred)

    s_full = np.zeros((kp, bs, ng), dtype=np.float16)
    t_full = np.zeros((kp, bs, npr), dtype=np.float16)
    s_full[0], t_full[0] = g2h, 1.0
    s_full[1], t_full[1] = g2l, 1.0
    s_full[2], t_full[2] = 1.0, p2h
    s_full[3], t_full[3] = 1.0, p2l
    for d in range(3):
        s_full[4 + d], t_full[4 + d] = mh[..., d], ph[..., d]
        s_full[7 + d], t_full[7 + d] = ml[..., d], ph[..., d]
        s_full[10 + d], t_full[10 + d] = mh[..., d], pl[..., d]
    s_full = -s_full  # device computes -d2; mins become maxes
    s_rep = np.zeros((128, bs, ng), dtype=np.float16)
    t_rep = np.zeros((128, bs, npr), dtype=np.float16)
    for q in range(4):
        s_rep[32 * q : 32 * q + kp] = s_full
        t_rep[32 * q : 32 * q + kp] = t_full
    return s_rep, t_rep


_NC_CACHE = {}


def _get_nc():
    key = (PACKED, WARMUP, TAIL0, TAIL1)
    if key not in _NC_CACHE:
        _NC_CACHE[key] = build_nc()
    return _NC_CACHE[key]


def _run_device(s_full, t_full, run_kwargs=None):
    from concourse.bass_utils import run_bass_kernel_spmd

    nc = _get_nc()
    in_maps = [
        {
            "s_in": np.ascontiguousarray(s_full[:, :, c * OWN : (c + 1) * OWN]),
            "t_in": t_full,
        }
        for c in range(N_CORES)
    ]
    res = run_bass_kernel_spmd(
        nc, in_maps, core_ids=list(range(N_CORES)), **(run_kwargs or {})
    )
    return res


def _combine(results):
    neg_d1 = np.empty((B, N), dtype=np.float32)
    colmaxes = []
    for c, out in enumerate(results):
        rm = out["rowmax_out"].transpose(1, 2, 0).reshape(B, OWN)
        neg_d1[:, c * OWN : (c + 1) * OWN] = rm
        cms = []
        for b in range(B):
            if (TAIL0 if b == 0 else TAIL1) == "par":
                cms.append(out["colmax_par"][b].astype(np.float32))
            else:
                # colmax_tr [b, 128(p), 64]: pred idx = col*128 + p
                cms.append(out["colmax_tr"][b].transpose(1, 0).reshape(N))
        colmaxes.append(np.stack(cms))
    neg_d2 = np.max(np.stack(colmaxes, axis=0), axis=0)
    d1 = np.sqrt(np.maximum(-neg_d1.astype(np.float64), 0.0))
    d2 = np.sqrt(np.maximum(-neg_d2.astype(np.float64), 0.0))
    val = np.mean(np.mean(d1, axis=1) + np.mean(d2, axis=1))
    return np.float32(val)


def kernel(pred: np.ndarray, gt: np.ndarray) -> np.ndarray:
    s_full, t_full = _pack_inputs(pred, gt)
    res = _run_device(s_full, t_full)
    return _combine(res.results)
